# revision 3
# baseline (speedup 1.0000x reference)
"""Transformer block (QKV + causal MHA + proj + GELU-FF, residual) on 8 NeuronCores.

Sharding: DP over batch (2 groups of 4 cores) x TP over heads / FF-inner within
each group. Identical SPMD program on all cores; per-core differences are input
slices only. Activations are kept feature-major ("transposed") end to end so no
on-device transposes are needed. Matmuls run in float32r (full PE rate at
free-dim >= 256, ~2^-13 rounding). proj and ff2 partials accumulate in the same
PSUM group; each token chunk is ReduceScattered across the 4-core group while
the next chunk computes. Host adds x + b_ff2 (residual) during unshard.

SBUF plan (KB per partition, cap 192):
  P1a: cst 8.4 + attnT 32 + qk 64 + v 32 + xc 32 + wqk-stream 16       = 184
  P1b: cst 8.4 + attnT 32 + qk 64 + v 32 + xcq-stream 16 + wv 32      = 184
  P2:  cst 8.4 + attnT 32 + qk 64 + v 32 + work ~12                   = 148
  P3:  cst 8.4 + attnT 32 + wp 32 + xc 64 + h 32 + w-streams 32 + o 4 = 204->188
PSUM: pmm(2) + po(2) + psums(2) + pbc(1) = 7 banks of 8.
"""
import numpy as np

import concourse.bass as bass
import concourse.mybir as mybir
import concourse.tile as tile
from concourse import bacc
from concourse import bass_utils

# problem dims (hardcoded per the task contract)
B, T, C = 2, 2048, 2048
H, HD = 16, 128
F = 8192
NCORES = 8
TPG = 4                  # cores per batch group
HPC = H // TPG           # heads per core (4)
QC = 4                   # token chunks per batch
TCH = T // QC            # 512 tokens per chunk
KT = C // 128            # 16 contraction tiles over C
FPC = F // TPG           # ff rows per core (2048)
FT = FPC // 128          # 16 ff row tiles per core
COT = C // 128           # 16 output-channel tiles
SM_SCALE = 1.0 / float(np.sqrt(HD))
NEG = -60000.0

f32r = mybir.dt.float32r
f32 = mybir.dt.float32

_CACHED_NC = None


def build_nc():
    nc = bacc.Bacc("TRN2", target_bir_lowering=False, debug=False,
                   num_devices=NCORES)
    xT_t = nc.dram_tensor("xT", [C, T], f32r, kind="ExternalInput").ap()
    wqk_t = nc.dram_tensor("wqk", [C, 2 * HPC * HD], f32r, kind="ExternalInput").ap()
    wv_t = nc.dram_tensor("wv", [C, HPC * HD], f32r, kind="ExternalInput").ap()
    wp_t = nc.dram_tensor("wp", [HPC * HD, C], f32r, kind="ExternalInput").ap()
    w1_t = nc.dram_tensor("w1", [C, FPC], f32r, kind="ExternalInput").ap()
    b1_t = nc.dram_tensor("b1", [128, FT], f32, kind="ExternalInput").ap()
    w2_t = nc.dram_tensor("w2", [FPC, C], f32r, kind="ExternalInput").ap()
    out_t = nc.dram_tensor("outp", [C // TPG, T], f32, kind="ExternalOutput").ap()

    xT_v = xT_t.rearrange("(kt p) t -> p kt t", p=128)      # [128, KT, T]
    wqk_v = wqk_t.rearrange("(kt p) f -> p kt f", p=128)    # [128, KT, 1024]
    wv_v = wv_t.rearrange("(kt p) f -> p kt f", p=128)      # [128, KT, 512]
    wp_v = wp_t.rearrange("(kt p) c -> p kt c", p=128)      # [128, 4, C]
    w1_v = w1_t.rearrange("(kt p) f -> p kt f", p=128)      # [128, KT, FPC]
    w2_v = w2_t.rearrange("(ft p) c -> p ft c", p=128)      # [128, FT, C]

    with tile.TileContext(nc) as tc:
        with tc.tile_pool(name="cstp", bufs=1) as cst, \
             tc.tile_pool(name="attp", bufs=1) as attp, \
             tc.tile_pool(name="ps", bufs=1, space="PSUM") as ps, \
             tc.tile_pool(name="dram", bufs=1, space="DRAM") as dram:

            # ---- constants ----
            ones_col = cst.tile([128, 1], f32r, name="ones_col", tag="oc")
            nc.gpsimd.memset(ones_col[:].bitcast(f32), 1.0)
            ones_row = cst.tile([1, 128], f32r, name="ones_row", tag="or")
            nc.gpsimd.memset(ones_row[:].bitcast(f32), 1.0)
            # causal masks: masks[:, d, q] for key tile at in-chunk offset d
            masks = cst.tile([128, QC, TCH], f32, name="masks", tag="mask")
            nc.gpsimd.memset(masks[:], 0.0)
            for d in range(QC):
                nc.gpsimd.affine_select(
                    out=masks[:, d, :], in_=masks[:, d, :],
                    compare_op=mybir.AluOpType.is_ge,
                    fill=NEG, base=-d * 128,
                    pattern=[[1, TCH]], channel_multiplier=-1,
                )
            b1_sb = cst.tile([128, FT], f32, name="b1_sb", tag="b1")
            nc.sync.dma_start(b1_sb[:], b1_t)

            # attn output persists into P3 (allocated early: pool LIFO order)
            attnT = attp.tile([128, HPC, T], f32r, name="attnT", tag="attnT")

            with tc.tile_pool(name="p12", bufs=1) as p12:
                qk_sb = p12.tile([128, 2 * HPC, QC, TCH], f32r, name="qk_sb",
                                 tag="qk")
                v_sb = p12.tile([128, T // 128, HPC * HD], f32r, name="v_sb",
                                tag="v")

                # ---- P1a: qT/kT = w_qk^T @ x  (feature-major) ----
                with tc.tile_pool(name="p1aw", bufs=1) as p1aw:
                    for c in range(QC):
                        xc = p1aw.tile([128, KT, TCH], f32r, name="xc",
                                       tag="xc", bufs=1)
                        nc.sync.dma_start(xc[:], xT_v[:, :, c * TCH:(c + 1) * TCH])
                        for ft in range(2 * HPC):
                            wqkt = p1aw.tile([128, KT, 128], f32r, name="wqkt",
                                             tag="wqkt", bufs=2)
                            nc.sync.dma_start(
                                wqkt[:], wqk_v[:, :, ft * 128:(ft + 1) * 128])
                            pt = ps.tile([128, TCH], f32, name="pmm", tag="pmm",
                                         bufs=2)
                            for k in range(KT):
                                nc.tensor.matmul(
                                    pt[:], wqkt[:, k, :], xc[:, k, :],
                                    start=(k == 0), stop=(k == KT - 1))
                            nc.vector.tensor_copy(qk_sb[:, ft, c, :], pt[:])

                # ---- P1b: v = x @ w_v  (token-major) ----
                with tc.tile_pool(name="p1bw", bufs=1) as p1bw:
                    wv_sb = p1bw.tile([128, KT, HPC * HD], f32r, name="wv_sb",
                                      tag="wv", bufs=1)
                    nc.sync.dma_start(wv_sb[:], wv_v)
                    for c in range(QC):
                        for m in range(TCH // 128):
                            xcq = p1bw.tile([128, KT, 128], f32r, name="xcq",
                                            tag="xcq", bufs=2)
                            nc.sync.dma_start(
                                xcq[:],
                                xT_v[:, :, c * TCH + m * 128:c * TCH + (m + 1) * 128])
                            pt = ps.tile([128, HPC * HD], f32, name="pmm",
                                         tag="pmm", bufs=2)
                            for k in range(KT):
                                nc.tensor.matmul(
                                    pt[:], xcq[:, k, :], wv_sb[:, k, :],
                                    start=(k == 0), stop=(k == KT - 1))
                            nc.vector.tensor_copy(
                                v_sb[:, c * (TCH // 128) + m, :], pt[:])

                # ---- P2: causal attention for this core's HPC heads ----
                with tc.tile_pool(name="p2w", bufs=1) as p2w:
                    for h in range(HPC):
                        qT_h = qk_sb[:, h, :, :]            # [128, QC, TCH]
                        kT_h = qk_sb[:, HPC + h, :, :]      # [128, QC, TCH]
                        for c in range(QC):
                            po = ps.tile([128, TCH], f32, name="po", tag="po",
                                         bufs=2)
                            psums = ps.tile([1, TCH], f32, name="psums",
                                            tag="psums", bufs=2)
                            nkt = 4 * c + 4
                            for kt in range(nkt):
                                pscore = ps.tile([128, TCH], f32, name="pmm",
                                                 tag="pmm", bufs=2)
                                nc.tensor.matmul(
                                    pscore[:],
                                    kT_h[:, kt // 4,
                                         (kt % 4) * 128:(kt % 4 + 1) * 128],
                                    qT_h[:, c, :], start=True, stop=True)
                                e_sb = p2w.tile([128, TCH], f32r, name="e_sb",
                                                tag="e", bufs=3)
                                if kt >= 4 * c:
                                    d = kt - 4 * c
                                    ms = p2w.tile([128, TCH], f32, name="ms",
                                                  tag="ms", bufs=2)
                                    nc.vector.tensor_add(ms[:], pscore[:],
                                                         masks[:, d, :])
                                    nc.scalar.activation(
                                        e_sb[:], ms[:],
                                        mybir.ActivationFunctionType.Exp,
                                        scale=SM_SCALE)
                                else:
                                    nc.scalar.activation(
                                        e_sb[:], pscore[:],
                                        mybir.ActivationFunctionType.Exp,
                                        scale=SM_SCALE)
                                nc.tensor.matmul(
                                    psums[:], ones_col[:], e_sb[:],
                                    start=(kt == 0), stop=(kt == nkt - 1))
                                nc.tensor.matmul(
                                    po[:], v_sb[:, kt, h * HD:(h + 1) * HD],
                                    e_sb[:], start=(kt == 0),
                                    stop=(kt == nkt - 1))
                            recip = p2w.tile([1, TCH], f32r, name="recip",
                                             tag="recip", bufs=2)
                            with nc.allow_low_precision(
                                    reason="f32r softmax 1/sum, 2^-13 ok"):
                                nc.vector.reciprocal(recip[:], psums[:])
                            pbc = ps.tile([128, TCH], f32, name="pbc", tag="pbc",
                                          bufs=1)
                            nc.tensor.matmul(pbc[:], ones_row[:], recip[:],
                                             start=True, stop=True)
                            bc_sb = p2w.tile([128, TCH], f32, name="bc_sb",
                                             tag="bc", bufs=2)
                            nc.vector.tensor_copy(bc_sb[:], pbc[:])
                            nc.vector.tensor_mul(
                                attnT[:, h, c * TCH:(c + 1) * TCH],
                                po[:], bc_sb[:])

            # ---- P3: ff1+gelu, then proj+ff2 into one PSUM, chunked RS ----
            with tc.tile_pool(name="p3w", bufs=1) as p3w:
                wp_sb = p3w.tile([128, TPG, C], f32r, name="wp_sb", tag="wp",
                                 bufs=1)
                nc.sync.dma_start(wp_sb[:], wp_v)
                for c in range(QC):
                    xc = p3w.tile([128, KT, TCH], f32r, name="xc3", tag="xc3",
                                  bufs=2)
                    nc.sync.dma_start(xc[:], xT_v[:, :, c * TCH:(c + 1) * TCH])
                    h_sb = p3w.tile([128, FT, TCH], f32r, name="h_sb", tag="h",
                                    bufs=1)
                    for f in range(FT):
                        w1tile = p3w.tile([128, KT, 128], f32r, name="w1tile",
                                          tag="w1t", bufs=2)
                        nc.sync.dma_start(w1tile[:],
                                          w1_v[:, :, f * 128:(f + 1) * 128])
                        ph = ps.tile([128, TCH], f32, name="pmm", tag="pmm",
                                     bufs=2)
                        for k in range(KT):
                            nc.tensor.matmul(ph[:], w1tile[:, k, :], xc[:, k, :],
                                             start=(k == 0), stop=(k == KT - 1))
                        nc.scalar.activation(h_sb[:, f, :], ph[:],
                                             mybir.ActivationFunctionType.Gelu,
                                             bias=b1_sb[:, f:f + 1], scale=1.0)
                    rs_in = dram.tile([COT * 128, TCH], f32, name="rs_in",
                                      tag="rsi", bufs=2)
                    for co in range(COT):
                        w2tile = p3w.tile([128, FT, 128], f32r, name="w2tile",
                                          tag="w2t", bufs=2)
                        nc.sync.dma_start(w2tile[:],
                                          w2_v[:, :, co * 128:(co + 1) * 128])
                        pout = ps.tile([128, TCH], f32, name="pmm", tag="pmm",
                                       bufs=2)
                        for k4 in range(TPG):
                            nc.tensor.matmul(
                                pout[:], wp_sb[:, k4, co * 128:(co + 1) * 128],
                                attnT[:, k4, c * TCH:(c + 1) * TCH],
                                start=(k4 == 0), stop=False)
                        for f in range(FT):
                            nc.tensor.matmul(pout[:], w2tile[:, f, :],
                                             h_sb[:, f, :],
                                             start=False, stop=(f == FT - 1))
                        o_sb = p3w.tile([128, TCH], f32, name="o_sb", tag="o",
                                        bufs=2)
                        nc.vector.tensor_copy(o_sb[:], pout[:])
                        nc.sync.dma_start(rs_in[co * 128:(co + 1) * 128, :],
                                          o_sb[:])
                    rs_out = dram.tile([(COT * 128) // TPG, TCH], f32,
                                       name="rs_out", tag="rso", bufs=2)
                    nc.gpsimd.collective_compute(
                        "ReduceScatter", mybir.AluOpType.add,
                        replica_groups=[[0, 1, 2, 3], [4, 5, 6, 7]],
                        ins=[rs_in.opt()], outs=[rs_out.opt()])
                    nc.sync.dma_start(out_t[:, c * TCH:(c + 1) * TCH], rs_out[:])

    nc.compile()
    return nc


def make_in_maps(x, w_qkv, w_proj, w_ff1, b_ff1, w_ff2):
    in_maps = []
    asc = np.ascontiguousarray
    for r in range(NCORES):
        b, hg = r // TPG, r % TPG
        q_cols = w_qkv[:, hg * 512:(hg + 1) * 512]
        k_cols = w_qkv[:, C + hg * 512:C + (hg + 1) * 512]
        v_cols = w_qkv[:, 2 * C + hg * 512:2 * C + (hg + 1) * 512]
        in_maps.append({
            "xT": asc(x[b].T),
            "wqk": asc(np.concatenate([q_cols, k_cols], axis=1)),
            "wv": asc(v_cols),
            "wp": asc(w_proj[hg * 512:(hg + 1) * 512, :]),
            "w1": asc(w_ff1[:, hg * FPC:(hg + 1) * FPC]),
            "b1": asc(b_ff1[hg * FPC:(hg + 1) * FPC].reshape(FT, 128).T),
            "w2": asc(w_ff2[hg * FPC:(hg + 1) * FPC, :]),
        })
    return in_maps


def assemble(results, x, b_ff2):
    out = np.empty((B, T, C), np.float32)
    for r in range(NCORES):
        b, idx = r // TPG, r % TPG
        # RS rank idx of each group holds couts [512*idx, 512*(idx+1))
        out[b, :, idx * 512:(idx + 1) * 512] = results[r]["outp"].T
    out += x + b_ff2
    return out


def kernel(x, w_qkv, w_proj, w_ff1, b_ff1, w_ff2, b_ff2):
    global _CACHED_NC
    x = np.asarray(x, np.float32)
    if _CACHED_NC is None:
        _CACHED_NC = build_nc()
    in_maps = make_in_maps(x, np.asarray(w_qkv, np.float32),
                           np.asarray(w_proj, np.float32),
                           np.asarray(w_ff1, np.float32),
                           np.asarray(b_ff1, np.float32),
                           np.asarray(w_ff2, np.float32))
    res = bass_utils.run_bass_kernel_spmd(_CACHED_NC, in_maps,
                                          core_ids=list(range(NCORES)))
    return assemble(res.results, x, np.asarray(b_ff2, np.float32))
